# revision 4
# baseline (speedup 1.0000x reference)
"""LIF spike kernel (Trainium2, 8 cores) — eigenbasis decomposition.

Reference recurrence per element (B*N independent chains over T steps):
    mem' = 0.5*mem + x_t - w;  s = 1[mem' > 0.5]
    w'   = 0.9*w + 0.05*mem' + 0.05*s;  mem'' = mem' - 0.5*s

The (mem, w) linear part has eigenvalues 0.75 / 0.6. Using the left
eigenvectors (p = u + 6w spike-free, u = pre-reset potential) the whole
system reduces to:
    Ahat_{t+1} = 0.75*Ahat_t - 0.15*x_{t+1}     (Ahat := -0.15*(u + 6w))
    E_t        = x_{t+1} + Ahat_t               (plain add)
    B_{t+1}    = 0.6*B_t + E_t - 0.3*1[B_t > 0.5]   (B := u)
    s_t        = 1[B_t > 0.5]
    Ahat_0 = -0.15*x_0, B_0 = x_0
(verified bitwise-identical to the reference on hardware).

Engine schedule per step: DVE: Ahat custom op + B custom op; Pool (GpSimd):
E = tensor_tensor add (walrus rejects scaled ops on Pool); Act: Sign(B-0.5)
-> int8. The linear side (Ahat, E) is emitted two steps ahead of the
sequential B chain so every cross-engine edge has a full step of slack.
Sharding: batch*feature elements split evenly across 8 cores (the T
recurrence is elementwise, no communication). x is staged [T, E] per core
(2KB-contiguous DMA lines); spikes return as int8 [T, E], host maps >0.
"""

import numpy as np

import concourse.bass as bass
import concourse.bacc as bacc
import concourse.mybir as mybir
import concourse.tile as tile
from concourse.bass_utils import run_bass_kernel_spmd

import concourse.dve_ops as dops
from concourse.dve_ops import DveOp
from concourse.dve_spec import Spec, Src0, Src1, C0, C1, C2, lower
from concourse.dve_ops import has_src1
from concourse.dve_uop import DveOpSpec

B, N, T = 64, 8192, 100
N_CORES = 8
P = 128

F32 = mybir.dt.float32
F16 = mybir.dt.float16
I8 = mybir.dt.int8
Alu = mybir.AluOpType
Act = mybir.ActivationFunctionType


def _register(name, spec):
    for o in dops.OPS:
        if o.name == name:
            return o
    opcode = dops._CUSTOM_DVE_ROW_BASE + len(dops.OPS)
    assert opcode < 0x20
    shas = {}
    for ver in ("v3", "v4"):
        dspec = DveOpSpec(
            name=name, opcode=opcode, uops=lower(spec, ver=ver),
            rd1_en=has_src1(spec),
        )
        shas[ver] = dspec.sha(ver)
    op = DveOp(name, spec, subdim=False, uops_sha=shas)
    dops.OPS.append(op)
    dops._SUB_OPCODE_FOR_NAME[name] = opcode
    dops.CUSTOM_DVE_SPECS[name] = spec
    return op


# B' = s0*in0 - s1*(in0 > imm2) + in1
LIF_U = _register(
    "LIF_U_ANT",
    Spec(
        body=Src0 * C0 - (Src0 > C2) * C1 + Src1,
        reference=lambda in0, in1, s0, s1, imm2: in0 * s0
        - (in0 > imm2).astype(np.float32) * s1
        + in1,
    ),
)

# Ahat' = s0*in0 - s1*in1
LIF_A = _register(
    "LIF_A_ANT",
    Spec(
        body=Src0 * C0 - Src1 * C1,
        reference=lambda in0, in1, s0, s1: in0 * s0 - in1 * s1,
    ),
)


def chunk_plan(T_: int):
    """Ramped x-chunk sizes: fp16 input DMA delivers ~0.45us/step while the
    engines burn ~1.2us/step, so the ramp can double per chunk."""
    if T_ == 100:
        return [3, 6, 12, 24, 24, 24, 7]
    out = []
    t = 0
    while t < T_:
        n = min(20, T_ - t)
        out.append(n)
        t += n
    return out


def s_block_plan(T_: int):
    if T_ == 100:
        return [10] * 9 + [5, 3, 2]
    out = []
    t = 0
    while t < T_:
        n = min(10, T_ - t)
        out.append(n)
        t += n
    return out


def build_nc(T_: int, P_: int, F_: int, reps: int = 1):
    """reps > 1 repeats the whole computation inside one NEFF (same input,
    same output) — used only for repeat-differencing timing."""
    nc = bacc.Bacc("TRN2", target_bir_lowering=False, debug=False)
    E = P_ * F_
    chunks = chunk_plan(T_)
    starts = [sum(chunks[:i]) for i in range(len(chunks))]
    n_ch = len(chunks)
    max_ch = max(chunks)
    x_d = nc.dram_tensor("x", [T_, E], F16, kind="ExternalInput").ap()
    s_d = nc.dram_tensor("s", [T_, E], I8, kind="ExternalOutput").ap()

    def chunk_of(t):
        for i in range(n_ch):
            if t < starts[i] + chunks[i]:
                return i
        raise ValueError(t)

    s_blocks = s_block_plan(T_)
    s_starts = [sum(s_blocks[:i]) for i in range(len(s_blocks))]
    max_sb = max(s_blocks)

    def s_block_of(t):
        for i in range(len(s_blocks)):
            if t < s_starts[i] + s_blocks[i]:
                return i
        raise ValueError(t)

    with tile.TileContext(nc) as tc:
        with (
            tc.tile_pool(name="xp", bufs=3) as xp,
            tc.tile_pool(name="sp", bufs=2) as sp,
            tc.tile_pool(name="apool", bufs=4) as ap_pool,
            tc.tile_pool(name="bp", bufs=3) as bp,
            tc.tile_pool(name="ep", bufs=4) as ep,
            tc.tile_pool(name="zp", bufs=1) as zp,
        ):
          bias_m05 = zp.tile([P_, 1], F32, tag="b05")
          nc.gpsimd.memset(bias_m05[:], -0.5)
          for _rep in range(reps):
            x_tiles = {}
            a_tiles = {}   # t -> Ahat_t
            e_tiles = {}   # t -> E_t
            loaded = [-1]

            def load_chunk(k):
                n_t = chunks[k]
                xt = xp.tile([P_, max_ch * F_], F16, tag="x")
                dst = xt[:, :n_t * F_]
                src = x_d[starts[k]:starts[k] + n_t].rearrange(
                    "t (p f) -> p t f", p=P_
                )
                nc.sync.dma_start(
                    dst.rearrange("p (t f) -> p t f", t=n_t), src
                )
                x_tiles[k] = xt

            def ensure_chunk(k):
                while loaded[0] < k:
                    loaded[0] += 1
                    load_chunk(loaded[0])

            def x_slice(t):
                k = chunk_of(t)
                ensure_chunk(k + 1 if t == starts[k] and k + 1 < n_ch else k)
                tl = t - starts[k]
                return x_tiles[k][:, tl * F_:(tl + 1) * F_]

            ensure_chunk(0)
            s_chunk = sp.tile([P_, max_sb * F_], I8, tag="s")

            def emit_a(t):
                """DVE: Ahat_t for 1 <= t <= T-2.
                t == 1: Ahat_1 = 0.75*(-0.15*x_0) - 0.15*x_1."""
                if not (1 <= t < T_ - 1):
                    return
                a_new = ap_pool.tile([P_, F_], F32, tag="a")
                if t == 1:
                    nc.vector._custom_dve(
                        LIF_A, out=a_new[:], in0=x_slice(0)[:],
                        in1=x_slice(1)[:], s0=-0.1125, s1=0.15,
                    )
                else:
                    nc.vector._custom_dve(
                        LIF_A, out=a_new[:], in0=a_tiles[t - 1][:],
                        in1=x_slice(t)[:], s0=0.75, s1=0.15,
                    )
                a_tiles[t] = a_new

            def emit_e(t):
                """E_t = x_{t+1} + Ahat_t; t == 0 on DVE (Ahat_0 not
                materialized), later steps on Pool."""
                if not (0 <= t < T_ - 1):
                    return
                e_new = ep.tile([P_, F_], F32, tag="e")
                if t == 0:
                    nc.vector.scalar_tensor_tensor(
                        e_new[:], x_slice(0)[:], -0.15, x_slice(1)[:],
                        op0=Alu.mult, op1=Alu.add,
                    )
                else:
                    nc.gpsimd.tensor_tensor(
                        e_new[:], x_slice(t + 1)[:], a_tiles[t][:], op=Alu.add
                    )
                e_tiles[t] = e_new

            # Prologue: fill the pipeline two steps deep.
            emit_a(1)
            emit_e(0)
            emit_a(2)
            emit_e(1)

            b_prev = None  # B_0 = x_0
            for t in range(T_):
                b_t = b_prev if t > 0 else x_slice(0)

                sb = s_block_of(t)
                sl = t - s_starts[sb]
                sg = s_chunk[:, sl * F_:(sl + 1) * F_]
                nc.scalar.activation(
                    sg[:], b_t[:], Act.Sign, bias=bias_m05[:], scale=1.0
                )

                if t + 1 < T_:
                    # DVE: B_{t+1} = 0.6*B_t - 0.3*(B_t > 0.5) + E_t
                    b_new = bp.tile([P_, F_], F32, tag="b")
                    nc.vector._custom_dve(
                        LIF_U, out=b_new[:], in0=b_t[:], in1=e_tiles.pop(t)[:],
                        s0=0.6, s1=0.3, imm2=0.5,
                    )
                    b_prev = b_new

                # run the linear side two steps ahead
                emit_a(t + 3)
                emit_e(t + 2)

                if sl == s_blocks[sb] - 1:
                    n_t = s_blocks[sb]
                    dst = s_d[s_starts[sb]:s_starts[sb] + n_t].rearrange(
                        "t (p f) -> p t f", p=P_
                    )
                    nc.sync.dma_start(
                        dst,
                        s_chunk[:, :n_t * F_].rearrange(
                            "p (t f) -> p t f", t=n_t
                        ),
                    )
                    if t + 1 < T_:
                        s_chunk = sp.tile([P_, max_sb * F_], I8, tag="s")
    nc.compile()
    return nc


def make_in_maps(x: np.ndarray):
    """Per-core inputs: x [B, N, T] -> 8 x {x: [T, E] float16}.

    fp16 input quantization flips 2683 of 52.4M spikes for the reference
    input distribution (rel err 1.4e-2, inside the 2e-2 gate) and halves
    the dominant input-DMA cost."""
    b, n, t_ = x.shape
    e_tot = b * n
    e = e_tot // N_CORES
    xt = np.swapaxes(np.asarray(x).reshape(N_CORES, e, t_), 1, 2).astype(
        np.float16
    )  # [8, T, e], contiguous
    return [{"x": xt[c]} for c in range(N_CORES)]


def assemble_output(s_cores, b, n, t_):
    """8 x int8 [T, E] spike signs -> [B, N, T] float32 {0,1}."""
    e = (b * n) // N_CORES
    out = np.empty((N_CORES, e, t_), np.float32)
    for c in range(N_CORES):
        # int8 transpose first (4x less data to shuffle than f32)
        st = np.ascontiguousarray(np.asarray(s_cores[c]).T)  # [e, T]
        out[c] = (st > 0).astype(np.float32)
    return out.reshape(b, n, t_)


# Cached compiled module + PJRT runner so repeated kernel() calls skip
# rebuild/recompile.
_CACHE = {}


def _get_runner(t_, f_):
    key = (t_, f_)
    if key in _CACHE:
        return _CACHE[key]
    import jax
    from jax.sharding import NamedSharding
    from concourse import bass2jax as b2j

    nc = build_nc(t_, P, f_)
    b2j.install_neuronx_cc_hook()
    partition_name = (
        nc.partition_id_tensor.name if nc.partition_id_tensor else None
    )
    in_names, out_names, out_avals, zero_outs = [], [], [], []
    for alloc in nc.m.functions[0].allocations:
        if not isinstance(alloc, mybir.MemoryLocationSet):
            continue
        name = alloc.memorylocations[0].name
        if alloc.kind == "ExternalInput":
            if name != partition_name:
                in_names.append(name)
        elif alloc.kind == "ExternalOutput":
            shape = tuple(alloc.tensor_shape)
            dtype = mybir.dt.np(alloc.dtype)
            out_names.append(name)
            out_avals.append(jax.core.ShapedArray(shape, dtype))
            zero_outs.append(np.zeros(shape, dtype))
    n_params = len(in_names)
    n_outs = len(out_avals)
    all_in_names = list(in_names) + list(out_names)
    if partition_name is not None:
        all_in_names.append(partition_name)

    def _body(*args):
        operands = list(args)
        if partition_name is not None:
            operands.append(b2j.partition_id_tensor())
        outs = b2j._bass_exec_p.bind(
            *operands,
            out_avals=tuple(out_avals),
            in_names=tuple(all_in_names),
            out_names=tuple(out_names),
            lowering_input_output_aliases=(),
            sim_require_finite=True,
            sim_require_nnan=True,
            nc=nc,
        )
        return tuple(outs)

    devices = jax.devices()[:N_CORES]
    mesh = b2j.Mesh(np.asarray(devices), ("core",))
    in_specs = (b2j.PartitionSpec("core"),) * (n_params + n_outs)
    out_specs = (b2j.PartitionSpec("core"),) * n_outs
    donate = tuple(range(n_params, n_params + n_outs))
    sharded = jax.jit(
        b2j.shard_map(
            _body, mesh=mesh, in_specs=in_specs, out_specs=out_specs,
            check_rep=False,
        ),
        donate_argnums=donate,
        keep_unused=True,
    )
    sharding = NamedSharding(mesh, b2j.PartitionSpec("core"))
    runner = dict(
        nc=nc, sharded=sharded, in_names=in_names, out_names=out_names,
        zero_outs=zero_outs, sharding=sharding,
    )
    _CACHE[key] = runner
    return runner


def _run_device(in_maps, t_, f_):
    import jax

    r = _get_runner(t_, f_)
    concat_in = [
        np.concatenate([np.asarray(m[name]) for m in in_maps], axis=0)
        for name in r["in_names"]
    ]
    args_dev = [jax.device_put(a, r["sharding"]) for a in concat_in]
    zeros_dev = [
        jax.device_put(
            np.zeros((N_CORES * z.shape[0], *z.shape[1:]), z.dtype),
            r["sharding"],
        )
        for z in r["zero_outs"]
    ]
    outs = r["sharded"](*args_dev, *zeros_dev)
    jax.block_until_ready(outs)
    per_core_shapes = [z.shape for z in r["zero_outs"]]
    return [
        {
            name: np.asarray(outs[i]).reshape(
                N_CORES, *per_core_shapes[i]
            )[c]
            for i, name in enumerate(r["out_names"])
        }
        for c in range(N_CORES)
    ]


def kernel(x: np.ndarray) -> np.ndarray:
    x = np.asarray(x)
    b, n, t_ = x.shape
    e = (b * n) // N_CORES
    f_ = e // P
    in_maps = make_in_maps(x)
    try:
        results = _run_device(in_maps, t_, f_)
    except Exception:
        # fallback: plain one-shot path
        nc = build_nc(t_, P, f_)
        bkr = run_bass_kernel_spmd(
            nc, in_maps, list(range(N_CORES)), trace=False
        )
        results = bkr.results
    return assemble_output([r["s"] for r in results], b, n, t_)


# revision 5
# speedup vs baseline: 1.4884x; 1.4884x over previous
"""LIF spike kernel (Trainium2, 8 cores) — eigenbasis decomposition.

Reference recurrence per element (B*N independent chains over T steps):
    mem' = 0.5*mem + x_t - w;  s = 1[mem' > 0.5]
    w'   = 0.9*w + 0.05*mem' + 0.05*s;  mem'' = mem' - 0.5*s

The (mem, w) linear part has eigenvalues 0.75 / 0.6. Using the left
eigenvectors (p = u + 6w spike-free, u = pre-reset potential) the whole
system reduces to:
    Ahat_{t+1} = 0.75*Ahat_t - 0.15*x_{t+1}     (Ahat := -0.15*(u + 6w))
    E_t        = x_{t+1} + Ahat_t               (plain add)
    B_{t+1}    = 0.6*B_t + E_t - 0.3*1[B_t > 0.5]   (B := u)
    s_t        = 1[B_t > 0.5]
    Ahat_0 = -0.15*x_0, B_0 = x_0
(verified bitwise-identical to the reference on hardware).

Engine schedule per step: DVE: Ahat custom op + B custom op; Pool (GpSimd):
E = tensor_tensor add (walrus rejects scaled ops on Pool); Act: Sign(B-0.5)
-> int8. The linear side (Ahat, E) is emitted two steps ahead of the
sequential B chain so every cross-engine edge has a full step of slack.
Sharding: batch*feature elements split evenly across 8 cores (the T
recurrence is elementwise, no communication). x is staged [T, E] per core
(2KB-contiguous DMA lines); spikes return as int8 [T, E], host maps >0.
"""

import numpy as np

import concourse.bass as bass
import concourse.bacc as bacc
import concourse.mybir as mybir
import concourse.tile as tile
from concourse.bass_utils import run_bass_kernel_spmd

import concourse.dve_ops as dops
from concourse.dve_ops import DveOp
from concourse.dve_spec import Spec, Src0, Src1, C0, C1, C2, lower
from concourse.dve_ops import has_src1
from concourse.dve_uop import DveOpSpec

B, N, T = 64, 8192, 100
N_CORES = 8
P = 128

F32 = mybir.dt.float32
F16 = mybir.dt.float16
I8 = mybir.dt.int8
Alu = mybir.AluOpType
Act = mybir.ActivationFunctionType


def _register(name, spec):
    for o in dops.OPS:
        if o.name == name:
            return o
    opcode = dops._CUSTOM_DVE_ROW_BASE + len(dops.OPS)
    assert opcode < 0x20
    shas = {}
    for ver in ("v3", "v4"):
        dspec = DveOpSpec(
            name=name, opcode=opcode, uops=lower(spec, ver=ver),
            rd1_en=has_src1(spec),
        )
        shas[ver] = dspec.sha(ver)
    op = DveOp(name, spec, subdim=False, uops_sha=shas)
    dops.OPS.append(op)
    dops._SUB_OPCODE_FOR_NAME[name] = opcode
    dops.CUSTOM_DVE_SPECS[name] = spec
    return op


# B' = s0*in0 - s1*(in0 > imm2) + in1
LIF_U = _register(
    "LIF_U_ANT",
    Spec(
        body=Src0 * C0 - (Src0 > C2) * C1 + Src1,
        reference=lambda in0, in1, s0, s1, imm2: in0 * s0
        - (in0 > imm2).astype(np.float32) * s1
        + in1,
    ),
)

# Ahat' = s0*in0 - s1*in1
LIF_A = _register(
    "LIF_A_ANT",
    Spec(
        body=Src0 * C0 - Src1 * C1,
        reference=lambda in0, in1, s0, s1: in0 * s0 - in1 * s1,
    ),
)


def chunk_plan(T_: int):
    """Ramped x-chunk sizes: fp16 input DMA delivers ~0.45us/step while the
    engines burn ~1.2us/step, so the ramp can double per chunk."""
    if T_ == 100:
        return [3, 6, 12, 24, 24, 24, 7]
    out = []
    t = 0
    while t < T_:
        n = min(20, T_ - t)
        out.append(n)
        t += n
    return out


def s_block_plan(T_: int):
    if T_ == 100:
        return [10] * 9 + [5, 3, 2]
    out = []
    t = 0
    while t < T_:
        n = min(10, T_ - t)
        out.append(n)
        t += n
    return out


def build_nc(T_: int, P_: int, F_: int, reps: int = 1):
    """reps > 1 repeats the whole computation inside one NEFF (same input,
    same output) — used only for repeat-differencing timing."""
    nc = bacc.Bacc("TRN2", target_bir_lowering=False, debug=False)
    E = P_ * F_
    chunks = chunk_plan(T_)
    starts = [sum(chunks[:i]) for i in range(len(chunks))]
    n_ch = len(chunks)
    max_ch = max(chunks)
    x_d = nc.dram_tensor("x", [T_, E], F16, kind="ExternalInput").ap()
    s_d = nc.dram_tensor("s", [T_, E], I8, kind="ExternalOutput").ap()

    def chunk_of(t):
        for i in range(n_ch):
            if t < starts[i] + chunks[i]:
                return i
        raise ValueError(t)

    s_blocks = s_block_plan(T_)
    s_starts = [sum(s_blocks[:i]) for i in range(len(s_blocks))]
    max_sb = max(s_blocks)

    def s_block_of(t):
        for i in range(len(s_blocks)):
            if t < s_starts[i] + s_blocks[i]:
                return i
        raise ValueError(t)

    with tile.TileContext(nc) as tc:
        with (
            tc.tile_pool(name="xp", bufs=3) as xp,
            tc.tile_pool(name="sp", bufs=2) as sp,
            tc.tile_pool(name="apool", bufs=4) as ap_pool,
            tc.tile_pool(name="bp", bufs=3) as bp,
            tc.tile_pool(name="ep", bufs=4) as ep,
            tc.tile_pool(name="zp", bufs=1) as zp,
        ):
          bias_m05 = zp.tile([P_, 1], F32, tag="b05")
          nc.gpsimd.memset(bias_m05[:], -0.5)
          for _rep in range(reps):
            x_tiles = {}
            a_tiles = {}   # t -> Ahat_t
            e_tiles = {}   # t -> E_t
            loaded = [-1]

            def load_chunk(k):
                n_t = chunks[k]
                xt = xp.tile([P_, max_ch * F_], F16, tag="x")
                dst = xt[:, :n_t * F_]
                src = x_d[starts[k]:starts[k] + n_t].rearrange(
                    "t (p f) -> p t f", p=P_
                )
                nc.sync.dma_start(
                    dst.rearrange("p (t f) -> p t f", t=n_t), src
                )
                x_tiles[k] = xt

            def ensure_chunk(k):
                while loaded[0] < k:
                    loaded[0] += 1
                    load_chunk(loaded[0])

            def x_slice(t):
                k = chunk_of(t)
                ensure_chunk(k + 1 if t == starts[k] and k + 1 < n_ch else k)
                tl = t - starts[k]
                return x_tiles[k][:, tl * F_:(tl + 1) * F_]

            ensure_chunk(0)
            s_chunk = sp.tile([P_, max_sb * F_], I8, tag="s")

            def emit_a(t):
                """DVE: Ahat_t for 1 <= t <= T-2.
                t == 1: Ahat_1 = 0.75*(-0.15*x_0) - 0.15*x_1."""
                if not (1 <= t < T_ - 1):
                    return
                # fp16 Ahat state: halves DVE/Pool operand traffic; adds 662
                # spike flips vs the fp32-state variant (total rel err 0.0156,
                # still under the 2e-2 gate — verified exactly in numpy).
                a_new = ap_pool.tile([P_, F_], F16, tag="a")
                if t == 1:
                    nc.vector._custom_dve(
                        LIF_A, out=a_new[:], in0=x_slice(0)[:],
                        in1=x_slice(1)[:], s0=-0.1125, s1=0.15,
                    )
                else:
                    nc.vector._custom_dve(
                        LIF_A, out=a_new[:], in0=a_tiles[t - 1][:],
                        in1=x_slice(t)[:], s0=0.75, s1=0.15,
                    )
                a_tiles[t] = a_new

            def emit_e(t):
                """E_t = x_{t+1} + Ahat_t; t == 0 on DVE (Ahat_0 not
                materialized), later steps on Pool."""
                if not (0 <= t < T_ - 1):
                    return
                e_new = ep.tile([P_, F_], F32, tag="e")
                if t == 0:
                    nc.vector.scalar_tensor_tensor(
                        e_new[:], x_slice(0)[:], -0.15, x_slice(1)[:],
                        op0=Alu.mult, op1=Alu.add,
                    )
                else:
                    nc.gpsimd.tensor_tensor(
                        e_new[:], x_slice(t + 1)[:], a_tiles[t][:], op=Alu.add
                    )
                e_tiles[t] = e_new

            # Prologue: fill the pipeline two steps deep.
            emit_a(1)
            emit_e(0)
            emit_a(2)
            emit_e(1)

            b_prev = None  # B_0 = x_0
            for t in range(T_):
                b_t = b_prev if t > 0 else x_slice(0)

                sb = s_block_of(t)
                sl = t - s_starts[sb]
                sg = s_chunk[:, sl * F_:(sl + 1) * F_]
                nc.scalar.activation(
                    sg[:], b_t[:], Act.Sign, bias=bias_m05[:], scale=1.0
                )

                if t + 1 < T_:
                    # DVE: B_{t+1} = 0.6*B_t - 0.3*(B_t > 0.5) + E_t
                    b_new = bp.tile([P_, F_], F32, tag="b")
                    nc.vector._custom_dve(
                        LIF_U, out=b_new[:], in0=b_t[:], in1=e_tiles.pop(t)[:],
                        s0=0.6, s1=0.3, imm2=0.5,
                    )
                    b_prev = b_new

                # run the linear side two steps ahead
                emit_a(t + 3)
                emit_e(t + 2)

                if sl == s_blocks[sb] - 1:
                    n_t = s_blocks[sb]
                    dst = s_d[s_starts[sb]:s_starts[sb] + n_t].rearrange(
                        "t (p f) -> p t f", p=P_
                    )
                    nc.sync.dma_start(
                        dst,
                        s_chunk[:, :n_t * F_].rearrange(
                            "p (t f) -> p t f", t=n_t
                        ),
                    )
                    if t + 1 < T_:
                        s_chunk = sp.tile([P_, max_sb * F_], I8, tag="s")
    nc.compile()
    return nc


def make_in_maps(x: np.ndarray):
    """Per-core inputs: x [B, N, T] -> 8 x {x: [T, E] float16}.

    fp16 input quantization flips 2683 of 52.4M spikes for the reference
    input distribution (rel err 1.4e-2, inside the 2e-2 gate) and halves
    the dominant input-DMA cost."""
    b, n, t_ = x.shape
    e_tot = b * n
    e = e_tot // N_CORES
    xt = np.swapaxes(np.asarray(x).reshape(N_CORES, e, t_), 1, 2).astype(
        np.float16
    )  # [8, T, e], contiguous
    return [{"x": xt[c]} for c in range(N_CORES)]


def assemble_output(s_cores, b, n, t_):
    """8 x int8 [T, E] spike signs -> [B, N, T] float32 {0,1}."""
    e = (b * n) // N_CORES
    out = np.empty((N_CORES, e, t_), np.float32)
    for c in range(N_CORES):
        # int8 transpose first (4x less data to shuffle than f32)
        st = np.ascontiguousarray(np.asarray(s_cores[c]).T)  # [e, T]
        out[c] = (st > 0).astype(np.float32)
    return out.reshape(b, n, t_)


# Cached compiled module + PJRT runner so repeated kernel() calls skip
# rebuild/recompile.
_CACHE = {}


def _get_runner(t_, f_):
    key = (t_, f_)
    if key in _CACHE:
        return _CACHE[key]
    import jax
    from jax.sharding import NamedSharding
    from concourse import bass2jax as b2j

    nc = build_nc(t_, P, f_)
    b2j.install_neuronx_cc_hook()
    partition_name = (
        nc.partition_id_tensor.name if nc.partition_id_tensor else None
    )
    in_names, out_names, out_avals, zero_outs = [], [], [], []
    for alloc in nc.m.functions[0].allocations:
        if not isinstance(alloc, mybir.MemoryLocationSet):
            continue
        name = alloc.memorylocations[0].name
        if alloc.kind == "ExternalInput":
            if name != partition_name:
                in_names.append(name)
        elif alloc.kind == "ExternalOutput":
            shape = tuple(alloc.tensor_shape)
            dtype = mybir.dt.np(alloc.dtype)
            out_names.append(name)
            out_avals.append(jax.core.ShapedArray(shape, dtype))
            zero_outs.append(np.zeros(shape, dtype))
    n_params = len(in_names)
    n_outs = len(out_avals)
    all_in_names = list(in_names) + list(out_names)
    if partition_name is not None:
        all_in_names.append(partition_name)

    def _body(*args):
        operands = list(args)
        if partition_name is not None:
            operands.append(b2j.partition_id_tensor())
        outs = b2j._bass_exec_p.bind(
            *operands,
            out_avals=tuple(out_avals),
            in_names=tuple(all_in_names),
            out_names=tuple(out_names),
            lowering_input_output_aliases=(),
            sim_require_finite=True,
            sim_require_nnan=True,
            nc=nc,
        )
        return tuple(outs)

    devices = jax.devices()[:N_CORES]
    mesh = b2j.Mesh(np.asarray(devices), ("core",))
    in_specs = (b2j.PartitionSpec("core"),) * (n_params + n_outs)
    out_specs = (b2j.PartitionSpec("core"),) * n_outs
    donate = tuple(range(n_params, n_params + n_outs))
    sharded = jax.jit(
        b2j.shard_map(
            _body, mesh=mesh, in_specs=in_specs, out_specs=out_specs,
            check_rep=False,
        ),
        donate_argnums=donate,
        keep_unused=True,
    )
    sharding = NamedSharding(mesh, b2j.PartitionSpec("core"))
    runner = dict(
        nc=nc, sharded=sharded, in_names=in_names, out_names=out_names,
        zero_outs=zero_outs, sharding=sharding,
    )
    _CACHE[key] = runner
    return runner


def _run_device(in_maps, t_, f_):
    import jax

    r = _get_runner(t_, f_)
    concat_in = [
        np.concatenate([np.asarray(m[name]) for m in in_maps], axis=0)
        for name in r["in_names"]
    ]
    args_dev = [jax.device_put(a, r["sharding"]) for a in concat_in]
    zeros_dev = [
        jax.device_put(
            np.zeros((N_CORES * z.shape[0], *z.shape[1:]), z.dtype),
            r["sharding"],
        )
        for z in r["zero_outs"]
    ]
    outs = r["sharded"](*args_dev, *zeros_dev)
    jax.block_until_ready(outs)
    per_core_shapes = [z.shape for z in r["zero_outs"]]
    return [
        {
            name: np.asarray(outs[i]).reshape(
                N_CORES, *per_core_shapes[i]
            )[c]
            for i, name in enumerate(r["out_names"])
        }
        for c in range(N_CORES)
    ]


def kernel(x: np.ndarray) -> np.ndarray:
    x = np.asarray(x)
    b, n, t_ = x.shape
    e = (b * n) // N_CORES
    f_ = e // P
    in_maps = make_in_maps(x)
    try:
        results = _run_device(in_maps, t_, f_)
    except Exception:
        # fallback: plain one-shot path
        nc = build_nc(t_, P, f_)
        bkr = run_bass_kernel_spmd(
            nc, in_maps, list(range(N_CORES)), trace=False
        )
        results = bkr.results
    return assemble_output([r["s"] for r in results], b, n, t_)
